# revision 29
# baseline (speedup 1.0000x reference)
"""Multi-Head Latent Attention (MLA) forward on 8 Trainium2 NeuronCores.

Contract: kernel(**inputs) takes the FULL unsharded inputs (numpy) and
returns the FULL [1, 4096, 2048] float32 output.

Sharding (hardcoded):
  - Tensor-parallel over heads: 2 heads per core (W_uk/W_uv/W_uq/W_qr/W_kr
    column-sharded, W_out row-sharded; partial outputs summed on host).
  - Down-projections (x @ W_dkv, x @ W_dq) are sharded over the sequence
    (512 rows per core) and AllGathered in bf16.

Device layout notes:
  - All activations are kept transposed ([dim, L]) so every matmul in the
    chain uses natural-layout weights and never needs an on-device transpose.
  - q/k head dims are reordered to [rope(64); base(64)] so the RoPE rows sit
    at partitions 0..63; the RoPE half-swap (x1,x2 -> x2,x1) is folded into
    extra "swapped" weight columns, so RoPE is pure lane-aligned elementwise.
  - Attention runs in the S^T = [Lk, Lq] orientation: softmax denominators
    are summed chunk-wise with a bf16 pair/quad tree + f32 accumulator on the
    DVE (no PE cycles), then reduced across partitions with one
    gpsimd.partition_all_reduce per (block, head).
  - Score and PV matmuls for the causally-dead column ranges of the 4
    diagonal chunks are skipped (the masked prefix of expS is memset to 0).
  - expS and the out-proj psum are double-buffered so consecutive
    (block, head) iterations and out-proj chunks pipeline.
  - Scores are small (|s| < ~2), so exp() runs without max-subtraction.
"""

import sys

for _p in ("/opt/trn_rl_repo", "/opt/pypackages"):
    if _p not in sys.path:
        sys.path.insert(0, _p)

import math
import numpy as np
import ml_dtypes

import concourse.bacc as bacc
import concourse.mybir as mybir
import concourse.tile as tile
from concourse.bass_isa import ReduceOp as _ReduceOp
from concourse.bass_utils import run_bass_kernel_spmd

# Problem constants
L = 4096
D = 2048
C = 512
H = 16
HD = 128          # head dim
ROPE = 64
HALF = ROPE // 2  # 32
SPLIT = HD - ROPE # 64
N_CORES = 8
HPC = H // N_CORES   # heads per core = 2
LLOC = L // N_CORES  # 512 (down-proj shard)
BQ = 512             # Lq block
NB = L // BQ         # 8
NKC = L // 128       # 32 Lk chunks
DKC = D // 128       # 16
CKC = C // 128       # 4
ROPE_BASE = 10000.0

BF16 = mybir.dt.bfloat16
F32 = mybir.dt.float32

_CACHE = {}

# Ablation flags for subtractive profiling (timing only; output garbage when set)
OPTS = {
    "no_attn": False,      # skip phase 3 entirely
    "no_outproj": False,   # skip phase 4
    "no_ag": False,        # skip the AllGather collectives
    "no_denom": False,     # skip ones-MMs + recip + broadcast (no normalize)
    "no_upproj": False,    # skip phase 2 (only valid with no_attn+no_outproj)
}


def _build_program(reps=1):
    nc = bacc.Bacc("TRN2", target_bir_lowering=False, debug=False, num_devices=N_CORES)

    xT = nc.dram_tensor("xT", [D, LLOC], BF16, kind="ExternalInput")
    wdkv = nc.dram_tensor("wdkv", [D, C], BF16, kind="ExternalInput")
    wdq = nc.dram_tensor("wdq", [D, C], BF16, kind="ExternalInput")
    wk = nc.dram_tensor("wk", [C, HPC * HD], BF16, kind="ExternalInput")
    wq = nc.dram_tensor("wq", [C, HPC * HD], BF16, kind="ExternalInput")
    wks = nc.dram_tensor("wks", [C, HPC * ROPE], BF16, kind="ExternalInput")
    wqs = nc.dram_tensor("wqs", [C, HPC * ROPE], BF16, kind="ExternalInput")
    wv = nc.dram_tensor("wv", [C, HPC * HD], BF16, kind="ExternalInput")
    # out-projection is column-sharded: full D rows, HPC*HD output columns
    wo = nc.dram_tensor("wo", [D, HPC * HD], BF16, kind="ExternalInput")
    CCd = nc.dram_tensor("CC", [128, L], BF16, kind="ExternalInput")
    SSd = nc.dram_tensor("SS", [128, L], BF16, kind="ExternalInput")
    CMd = nc.dram_tensor("CM", [4, 128, BQ], BF16, kind="ExternalInput")
    OUT = nc.dram_tensor("OUT", [L, HPC * HD], F32, kind="ExternalOutput")

    agin = nc.dram_tensor("agin", [2, C, LLOC], BF16)
    agout = nc.dram_tensor("agout", [N_CORES, 2, C, LLOC], BF16, addr_space="Shared")
    # per-q-block ctx AllGather buffers (ctxT is [2 heads x 128, BQ] bf16)
    cgin = [nc.dram_tensor(f"cgin{b}", [HPC * HD, BQ], BF16) for b in range(NB)]
    cgout = [
        nc.dram_tensor(f"cgout{b}", [N_CORES * HPC * HD, BQ], BF16, addr_space="Shared")
        for b in range(NB)
    ]

    rg = [list(range(N_CORES))]

    with tile.TileContext(nc) as tc:
        for _rep in range(reps):
            _emit_body(nc, tc, locals())
    nc.compile()
    return nc


def _emit_body(nc, tc, g):
    xT, wdkv, wdq = g["xT"], g["wdkv"], g["wdq"]
    wk, wq, wks, wqs, wv, wo = g["wk"], g["wq"], g["wks"], g["wqs"], g["wv"], g["wo"]
    CCd, SSd, CMd, OUT = g["CCd"], g["SSd"], g["CMd"], g["OUT"]
    agin, agout = g["agin"], g["agout"]
    cgin, cgout = g["cgin"], g["cgout"]
    rg = g["rg"]
    Exp = mybir.ActivationFunctionType.Exp
    Copy = mybir.ActivationFunctionType.Copy

    # Single shared PSUM pool, 8 banks:
    #   "s"   [128,2,BQ] x2 bufs = 4 banks   (S^T groups; down-proj psums)
    #   "ctx" [128,BQ]   x2 bufs = 2 banks   (PV accumulate; up-proj mains)
    #   "b1"  [128,BQ]   x1 buf  = 1 bank    (softmax denominator; V psums)
    #   "o"   [128,BQ]   x1 buf  = 1 bank    (out-proj psums; swap-rope psums)
    with tc.tile_pool(name="sb_base", bufs=1) as sbB, tc.tile_pool(
        name="ps", bufs=1, space="PSUM"
    ) as psP:
        kT = sbB.tile([128, HPC, L], BF16, tag="kT")
        qT = sbB.tile([128, HPC, L], BF16, tag="qT")
        vN = sbB.tile([128, NKC, HPC * HD], BF16, tag="vN")
        ctxT = sbB.tile([128, HPC, L], BF16, tag="ctxT")
        wo_t = sbB.tile([128, DKC, HPC * HD], BF16, tag="wo")
        cm_t = sbB.tile([128, 4, BQ], BF16, tag="cm")

        # ---------------- Phase 1: down-projections + AllGathers ------------
        with tc.tile_pool(name="sb_dp", bufs=1) as sbD:
            xT_t = sbD.tile([128, DKC, LLOC], BF16, tag="xT")
            wdkv_t = sbD.tile([128, DKC, C], BF16, tag="wdkv")
            wdq_t = sbD.tile([128, DKC, C], BF16, tag="wdq")
            xT4 = xT.rearrange("(k p) l -> p k l", p=128)
            wdkv4 = wdkv.rearrange("(k p) c -> p k c", p=128)
            wdq4 = wdq.rearrange("(k p) c -> p k c", p=128)
            for k in range(DKC):
                nc.sync.dma_start(xT_t[:, k, :], xT4[:, k, :])
                nc.sync.dma_start(wdkv_t[:, k, :], wdkv4[:, k, :])
            for k in range(DKC):
                nc.sync.dma_start(wdq_t[:, k, :], wdq4[:, k, :])

            for gi_, (which, w_t) in enumerate((("kv", wdkv_t), ("q", wdq_t))):
                stage = sbD.tile([128, CKC, LLOC], BF16, tag="cstage")
                for t in range(CKC):
                    ps = psP.tile([128, LLOC], F32, tag="s", bufs=2)
                    for k in range(DKC):
                        nc.tensor.matmul(
                            ps[:],
                            w_t[:, k, t * 128 : (t + 1) * 128],
                            xT_t[:, k, :],
                            start=(k == 0),
                            stop=(k == DKC - 1),
                        )
                    nc.vector.tensor_copy(stage[:, t, :], ps[:])
                nc.sync.dma_start(
                    agin[gi_].rearrange("(t p) l -> p t l", p=128), stage[:]
                )
                if gi_ == 1 and not OPTS["no_ag"]:
                    nc.gpsimd.collective_compute(
                        "AllGather",
                        mybir.AluOpType.bypass,
                        replica_groups=rg,
                        ins=[agin[:]],
                        outs=[agout[:]],
                    )


        # ---------------- Phase 2: up-projections + RoPE (per L-window) -----
        with tc.tile_pool(name="sb_up", bufs=1) as sbU:
            wk_t = sbU.tile([128, CKC, HPC * HD], BF16, tag="wk")
            wq_t = sbU.tile([128, CKC, HPC * HD], BF16, tag="wq")
            wks_t = sbU.tile([128, CKC, HPC * ROPE], BF16, tag="wks")
            wqs_t = sbU.tile([128, CKC, HPC * ROPE], BF16, tag="wqs")
            wv_t = sbU.tile([128, CKC, HPC * HD], BF16, tag="wv")
            nc.sync.dma_start(wk_t[:], wk.rearrange("(c p) m -> p c m", p=128))
            nc.sync.dma_start(wq_t[:], wq.rearrange("(c p) m -> p c m", p=128))
            nc.sync.dma_start(wks_t[:], wks.rearrange("(c p) m -> p c m", p=128))
            nc.sync.dma_start(wqs_t[:], wqs.rearrange("(c p) m -> p c m", p=128))
            nc.sync.dma_start(wv_t[:], wv.rearrange("(c p) m -> p c m", p=128))
            cc_t = sbU.tile([128, L], BF16, tag="cc")
            ss_t = sbU.tile([128, L], BF16, tag="ss")
            nc.sync.dma_start(cc_t[:], CCd[:])
            nc.sync.dma_start(ss_t[:], SSd[:])
            nc.sync.dma_start(wo_t[:], wo.rearrange("(k p) m -> p k m", p=128))
            nc.sync.dma_start(cm_t[:], CMd.rearrange("c p l -> p c l"))

            for w in range(NB):
                if OPTS["no_upproj"]:
                    break
                win = slice(w * BQ, (w + 1) * BQ)
                ckw = sbU.tile([128, CKC, BQ], BF16, tag="ckw", bufs=3)
                cqw = sbU.tile([128, CKC, BQ], BF16, tag="cqw", bufs=3)
                nc.sync.dma_start(
                    ckw[:], agout[w, 0].rearrange("(t p) l -> p t l", p=128)
                )
                nc.sync.dma_start(
                    cqw[:], agout[w, 1].rearrange("(t p) l -> p t l", p=128)
                )

                for dst, w_t, ws_t, src in (
                    (kT, wk_t, wks_t, ckw),
                    (qT, wq_t, wqs_t, cqw),
                ):
                    # swapped-rope projection, both heads in one psum:
                    # rows 0:64 = head0 swap (head0 rope rows), 64:128 = head1
                    ps_sw = psP.tile([128, BQ], F32, tag="o", bufs=2)
                    for c in range(CKC):
                        nc.tensor.matmul(
                            ps_sw[:],
                            ws_t[:, c, :],
                            src[:, c, :],
                            start=(c == 0),
                            stop=(c == CKC - 1),
                        )
                    swm = sbU.tile([128, BQ], BF16, tag="swm", bufs=2)
                    nc.vector.tensor_mul(swm[:], ps_sw[:], ss_t[:, win])

                    for h in range(HPC):
                        ps = psP.tile([128, BQ], F32, tag="ctx", bufs=2)
                        for c in range(CKC):
                            nc.tensor.matmul(
                                ps[:],
                                w_t[:, c, h * HD : (h + 1) * HD],
                                src[:, c, :],
                                start=(c == 0),
                                stop=(c == CKC - 1),
                            )
                        # head0 layout [rope;base]: rope rows 0:64
                        # head1 layout [base;rope]: rope rows 64:128
                        ro = slice(0, 64) if h == 0 else slice(64, 128)
                        ba = slice(64, 128) if h == 0 else slice(0, 64)
                        nc.scalar.activation(dst[ba, h, win], ps[ba, :], Copy)
                        nc.vector.tensor_mul(
                            dst[ro, h, win], ps[ro, :], cc_t[ro, win]
                        )
                        nc.vector.tensor_add(
                            dst[ro, h, win], dst[ro, h, win], swm[ro, :]
                        )

                # V: natural layout [Lk, d] chunks (both heads, N=256)
                for j in range(4):
                    lc = w * 4 + j
                    ps = psP.tile([128, BQ], F32, tag="o", bufs=2)
                    for c in range(CKC):
                        nc.tensor.matmul(
                            ps[:, 0 : HPC * HD],
                            ckw[:, c, j * 128 : (j + 1) * 128],
                            wv_t[:, c, :],
                            start=(c == 0),
                            stop=(c == CKC - 1),
                        )
                    nc.vector.tensor_copy(vN[:, lc, :], ps[:, 0 : HPC * HD])

        # -------- Phase 3: attention + fused out-projection (per q-block) ---
        with tc.tile_pool(name="sb_at", bufs=1) as sbA:
            # b=0 first (ready earliest), then largest blocks, smallest last
            # so the pipeline tail is short
            for b in (0, 7, 6, 5, 4, 3, 2, 1):
                if OPTS["no_attn"]:
                    break
                nch = 4 * (b + 1)
                qwin = slice(b * BQ, (b + 1) * BQ)
                for h in range(HPC):
                    expS = sbA.tile([128, NKC, BQ], BF16, tag="expS", bufs=2)
                    acc = sbA.tile([128, BQ], F32, tag="acc", bufs=2)
                    ctx_ps = psP.tile([128, BQ], F32, tag="ctx", bufs=2)
                    epair = None
                    # zero the dead prefixes of the 3 upper diagonal chunks so
                    # the denominator tree can read full-width rows
                    for dj in range(1, 4):
                        nc.vector.memset(expS[:, nch - 4 + dj, 0 : 128 * dj], 0.0)
                    ck0 = 0
                    while ck0 < nch:
                        gsz = min(2, nch - ck0)
                        s_ps = psP.tile([128, 2, BQ], F32, tag="s", bufs=2)
                        for j in range(gsz):
                            ck = ck0 + j
                            cs = 128 * (ck - (nch - 4)) if ck >= nch - 4 else 0
                            nc.tensor.matmul(
                                s_ps[:, j, cs:],
                                kT[:, h, ck * 128 : (ck + 1) * 128],
                                qT[:, h, b * BQ + cs : (b + 1) * BQ],
                                start=True,
                                stop=True,
                            )
                        cs0 = 128 * (ck0 - (nch - 4)) if ck0 >= nch - 4 else 0
                        if cs0 == 0 and (ck0 + gsz <= nch - 4 or ck0 == nch - 4):
                            # both chunks full-width (or first diag pair j0):
                            # j0 is full, j1 needs its own range
                            if ck0 + gsz <= nch - 4:
                                nc.scalar.activation(
                                    expS[:, ck0 : ck0 + gsz, :], s_ps[:, 0:gsz, :], Exp
                                )
                            else:
                                nc.scalar.activation(
                                    expS[:, ck0, :], s_ps[:, 0, :], Exp
                                )
                                nc.scalar.activation(
                                    expS[:, ck0 + 1, 128:], s_ps[:, 1, 128:], Exp
                                )
                        else:
                            for j in range(gsz):
                                ck = ck0 + j
                                cs = 128 * (ck - (nch - 4)) if ck >= nch - 4 else 0
                                nc.scalar.activation(
                                    expS[:, ck, cs:], s_ps[:, j, cs:], Exp
                                )
                        for j in range(gsz):
                            ck = ck0 + j
                            if ck >= nch - 4:  # diagonal chunk: causal mask
                                cs = 128 * (ck - (nch - 4))
                                nc.vector.tensor_mul(
                                    expS[:, ck, cs:],
                                    expS[:, ck, cs:],
                                    cm_t[:, ck - (nch - 4), cs:],
                                )
                        if not OPTS["no_denom"]:
                            # bf16 pair/quad tree + f32 quad accumulate, all
                            # on DVE (no PE cycles for the denominator)
                            if ck0 % 4 == 0:
                                epair = sbA.tile([128, BQ], BF16, tag="epair", bufs=2)
                                nc.vector.tensor_add(
                                    epair[:], expS[:, ck0, :], expS[:, ck0 + 1, :]
                                )
                            else:
                                equad = sbA.tile([128, BQ], BF16, tag="equad", bufs=2)
                                nc.vector.tensor_add(
                                    equad[:], expS[:, ck0, :], expS[:, ck0 + 1, :]
                                )
                                nc.vector.tensor_add(equad[:], equad[:], epair[:])
                                if ck0 == 2:
                                    nc.vector.tensor_copy(acc[:], equad[:])
                                else:
                                    nc.vector.tensor_add(acc[:], acc[:], equad[:])
                        for j in range(gsz):
                            ck = ck0 + j
                            cs = 128 * (ck - (nch - 4)) if ck >= nch - 4 else 0
                            nc.tensor.matmul(
                                ctx_ps[:, cs:],
                                vN[:, ck, h * HD : (h + 1) * HD],
                                expS[:, ck, cs:],
                                start=(ck == 0),
                                stop=(ck == nch - 1),
                                skip_group_check=(cs > 0),
                            )
                        ck0 += gsz
                    if OPTS["no_denom"]:
                        nc.vector.tensor_copy(ctxT[:, h, qwin], ctx_ps[:])
                        continue
                    dsum_t = sbA.tile([128, BQ], F32, tag="dsum_t", bufs=2)
                    nc.gpsimd.partition_all_reduce(dsum_t[:], acc[:], channels=128, reduce_op=_ReduceOp.add)
                    bc_t = sbA.tile([128, BQ], F32, tag="bc_t", bufs=2)
                    nc.vector.reciprocal_approx_fast(out=bc_t[:], in_=dsum_t[:])
                    nc.vector.tensor_mul(ctxT[:, h, qwin], ctx_ps[:], bc_t[:])

                # AllGather this block's ctx across cores (bf16, 256KB)
                nc.sync.dma_start(
                    cgin[b].rearrange("(h p) l -> p h l", p=128),
                    ctxT[:, :, qwin],
                )
                if not OPTS["no_ag"]:
                    nc.gpsimd.collective_compute(
                        "AllGather",
                        mybir.AluOpType.bypass,
                        replica_groups=rg,
                        ins=[cgin[b][:]],
                        outs=[cgout[b][:]],
                    )

                # column-sharded out-projection for this q-window: full 2048
                # ctx dims (all heads, all cores) x this core's 256 out cols
                if OPTS["no_outproj"]:
                    continue
                for j in range(4):
                    lc = b * 4 + j
                    call = sbA.tile([128, DKC, 128], BF16, tag="call", bufs=3)
                    nc.sync.dma_start(
                        call[:],
                        cgout[b].rearrange("(k p) l -> p k l", p=128)[
                            :, :, j * 128 : (j + 1) * 128
                        ],
                    )
                    ostage = sbA.tile([128, HPC * HD], F32, tag="ostage", bufs=3)
                    ps = psP.tile([128, BQ], F32, tag="o", bufs=2)
                    for k in range(DKC):
                        nc.tensor.matmul(
                            ps[:, 0 : HPC * HD],
                            call[:, k, :],
                            wo_t[:, k, :],
                            start=(k == 0),
                            stop=(k == DKC - 1),
                        )
                    nc.vector.tensor_copy(ostage[:], ps[:, 0 : HPC * HD])
                    nc.sync.dma_start(OUT[lc * 128 : (lc + 1) * 128, :], ostage[:])


def _host_inputs(x, W_dkv, W_dq, W_uk, W_uv, W_uq, W_qr, W_kr, W_out):
    """Build per-core input maps (numpy, bf16)."""
    bf = lambda a: np.ascontiguousarray(a).astype(ml_dtypes.bfloat16)
    scale = 1.0 / math.sqrt(HD)

    xt = np.ascontiguousarray(x.reshape(L, D).T)  # [D, L] f32

    # rope tables, transposed: ang[l, i] = l * inv_freq[i]
    inv_freq = 1.0 / (ROPE_BASE ** (np.arange(HALF, dtype=np.float64) * 2.0 / ROPE))
    ang = np.arange(L, dtype=np.float64)[:, None] * inv_freq[None, :]  # [L, 32]
    cosT = np.cos(ang).T.astype(np.float32)  # [32, L]
    sinT = np.sin(ang).T.astype(np.float32)
    cc64 = np.concatenate([cosT, cosT], axis=0)        # [64, L]
    ss64 = np.concatenate([-sinT, sinT], axis=0)       # [64, L]
    CC = np.concatenate([cc64, cc64], axis=0)          # [128, L] (both head slots)
    SS = np.concatenate([ss64, ss64], axis=0)          # [128, L]

    CM = np.zeros((4, 128, BQ), dtype=np.float32)
    for j in range(4):
        for lk in range(128):
            CM[j, lk, j * 128 + lk :] = 1.0

    in_maps = []
    for r in range(N_CORES):
        heads = [2 * r, 2 * r + 1]
        wk_blocks, wq_blocks, wks_blocks, wqs_blocks, wv_blocks = [], [], [], [], []
        for i, h in enumerate(heads):
            kr = W_kr[:, h * ROPE : (h + 1) * ROPE]
            kb = W_uk[:, h * SPLIT : (h + 1) * SPLIT]
            qr = W_qr[:, h * ROPE : (h + 1) * ROPE] * scale
            qb = W_uq[:, h * SPLIT : (h + 1) * SPLIT] * scale
            # head-dim order: slot0 head [rope;base], slot1 head [base;rope]
            # (rope rows land at partitions 0:64 / 64:128 respectively)
            if i == 0:
                wk_blocks.append(np.concatenate([kr, kb], axis=1))
                wq_blocks.append(np.concatenate([qr, qb], axis=1))
            else:
                wk_blocks.append(np.concatenate([kb, kr], axis=1))
                wq_blocks.append(np.concatenate([qb, qr], axis=1))
            # swapped rope halves for the rotate-half term (64 cols per head)
            wks_blocks.append(np.concatenate([kr[:, HALF:], kr[:, :HALF]], axis=1))
            wqs_blocks.append(np.concatenate([qr[:, HALF:], qr[:, :HALF]], axis=1))
            wv_blocks.append(W_uv[:, h * HD : (h + 1) * HD])
        # column shard of W_out (full rows) for this core's 256 output cols
        wo_cols = W_out[:, r * HPC * HD : (r + 1) * HPC * HD]
        in_maps.append(
            {
                "xT": bf(xt[:, r * LLOC : (r + 1) * LLOC]),
                "wdkv": bf(W_dkv),
                "wdq": bf(W_dq),
                "wk": bf(np.concatenate(wk_blocks, axis=1)),
                "wq": bf(np.concatenate(wq_blocks, axis=1)),
                "wks": bf(np.concatenate(wks_blocks, axis=1)),
                "wqs": bf(np.concatenate(wqs_blocks, axis=1)),
                "wv": bf(np.concatenate(wv_blocks, axis=1)),
                "wo": bf(wo_cols),
                "CC": bf(CC),
                "SS": bf(SS),
                "CM": bf(CM),
            }
        )
    return in_maps


def _get_program(reps=1):
    if reps not in _CACHE:
        _CACHE[reps] = _build_program(reps)
    return _CACHE[reps]


def make_runner(in_maps, reps=1, empty=False):
    """Persistent compiled runner for timing: returns run_chain(M) that executes
    the program M times back-to-back on device (chained via the output buffer so
    executions serialize), returning wall seconds for the chain."""
    import time as _time
    import jax
    from jax.sharding import Mesh, PartitionSpec, NamedSharding
    from jax.experimental.shard_map import shard_map
    import concourse.bass2jax as bass2jax

    nc = _EMPTY_CACHE.setdefault(0, _build_empty()) if empty else _get_program(reps)
    bass2jax.install_neuronx_cc_hook()
    partition_name = nc.partition_id_tensor.name if nc.partition_id_tensor else None
    in_names, out_names, out_avals, zero_outs = [], [], [], []
    for alloc in nc.m.functions[0].allocations:
        if not isinstance(alloc, mybir.MemoryLocationSet):
            continue
        name = alloc.memorylocations[0].name
        if alloc.kind == "ExternalInput":
            if name != partition_name:
                in_names.append(name)
        elif alloc.kind == "ExternalOutput":
            out_names.append(name)
            shape = tuple(alloc.tensor_shape)
            dtype = mybir.dt.np(alloc.dtype)
            out_avals.append(jax.core.ShapedArray(shape, dtype))
            zero_outs.append(np.zeros(shape, dtype))
    n_params = len(in_names)
    in_names_all = in_names + out_names
    if partition_name is not None:
        in_names_all = in_names_all + [partition_name]

    def _body(*args):
        operands = list(args)
        if partition_name is not None:
            operands.append(bass2jax.partition_id_tensor())
        outs = bass2jax._bass_exec_p.bind(
            *operands,
            out_avals=tuple(out_avals),
            in_names=tuple(in_names_all),
            out_names=tuple(out_names),
            lowering_input_output_aliases=(),
            sim_require_finite=True,
            sim_require_nnan=True,
            nc=nc,
        )
        return tuple(outs)

    devices = jax.devices()[:N_CORES]
    mesh = Mesh(np.asarray(devices), ("core",))
    n_outs = len(out_names)
    in_specs = (PartitionSpec("core"),) * (n_params + n_outs)
    out_specs = (PartitionSpec("core"),) * n_outs
    sharded = jax.jit(
        shard_map(_body, mesh=mesh, in_specs=in_specs, out_specs=out_specs, check_rep=False),
        keep_unused=True,
    )
    sh = NamedSharding(mesh, PartitionSpec("core"))
    concat_in = [
        np.concatenate([np.asarray(in_maps[c][nm]) for c in range(N_CORES)], axis=0)
        for nm in in_names
    ]
    concat_zeros = [
        np.zeros((N_CORES * z.shape[0], *z.shape[1:]), z.dtype) for z in zero_outs
    ]
    dev_in = [jax.device_put(a, sh) for a in concat_in]
    dev_zero = [jax.device_put(a, sh) for a in concat_zeros]
    outs = sharded(*dev_in, *dev_zero)
    jax.block_until_ready(outs)  # compile + warm

    def run_chain(M):
        z = list(dev_zero)
        t0 = _time.perf_counter()
        outs = None
        for _ in range(M):
            outs = sharded(*dev_in, *z)
            z = list(outs)
        jax.block_until_ready(outs)
        return _time.perf_counter() - t0

    return run_chain


_EMPTY_CACHE = {}


def _build_empty():
    """Minimal program with the same I/O signature class: one tiny DMA."""
    nc = bacc.Bacc("TRN2", target_bir_lowering=False, debug=False, num_devices=N_CORES)
    xT = nc.dram_tensor("xT", [128, 128], F32, kind="ExternalInput")
    OUT = nc.dram_tensor("OUT", [128, 128], F32, kind="ExternalOutput")
    with tile.TileContext(nc) as tc:
        with tc.tile_pool(name="sb", bufs=1) as sb:
            t = sb.tile([128, 128], F32, tag="t")
            nc.sync.dma_start(t[:], xT[:])
            nc.sync.dma_start(OUT[:], t[:])
    nc.compile()
    return nc


def make_empty_runner():
    in_maps = [{"xT": np.zeros((128, 128), np.float32)} for _ in range(N_CORES)]
    return make_runner(in_maps, empty=True)


def kernel(x, W_dkv, W_dq, W_uk, W_uv, W_uq, W_qr, W_kr, W_out, b_out, reps=1):
    x = np.asarray(x, dtype=np.float32)
    in_maps = _host_inputs(
        x,
        np.asarray(W_dkv, np.float32), np.asarray(W_dq, np.float32),
        np.asarray(W_uk, np.float32), np.asarray(W_uv, np.float32),
        np.asarray(W_uq, np.float32), np.asarray(W_qr, np.float32),
        np.asarray(W_kr, np.float32), np.asarray(W_out, np.float32),
    )
    nc = _get_program(reps)
    res = run_bass_kernel_spmd(nc, in_maps, core_ids=list(range(N_CORES)), trace=False)
    out = np.concatenate(
        [res.results[r]["OUT"] for r in range(N_CORES)], axis=1
    ).astype(np.float32)
    out += np.asarray(b_out, np.float32)[None, :]
    return out.reshape(1, L, D)



# revision 31
# speedup vs baseline: 1.0623x; 1.0623x over previous
"""Multi-Head Latent Attention (MLA) forward on 8 Trainium2 NeuronCores.

Contract: kernel(**inputs) takes the FULL unsharded inputs (numpy) and
returns the FULL [1, 4096, 2048] float32 output.

Sharding (hardcoded):
  - Tensor-parallel over heads: 2 heads per core (W_uk/W_uv/W_uq/W_qr/W_kr
    column-sharded, W_out row-sharded; partial outputs summed on host).
  - Down-projections (x @ W_dkv, x @ W_dq) are sharded over the sequence
    (512 rows per core) and AllGathered in bf16.

Device layout notes:
  - All activations are kept transposed ([dim, L]) so every matmul in the
    chain uses natural-layout weights and never needs an on-device transpose.
  - q/k head dims are reordered to [rope(64); base(64)] so the RoPE rows sit
    at partitions 0..63; the RoPE half-swap (x1,x2 -> x2,x1) is folded into
    extra "swapped" weight columns, so RoPE is pure lane-aligned elementwise.
  - Attention runs in the S^T = [Lk, Lq] orientation: softmax denominators
    are summed chunk-wise with a bf16 pair/quad tree + f32 accumulator on the
    DVE (no PE cycles), then reduced across partitions with one
    gpsimd.partition_all_reduce per (block, head).
  - Score and PV matmuls for the causally-dead column ranges of the 4
    diagonal chunks are skipped (the masked prefix of expS is memset to 0).
  - expS and the out-proj psum are double-buffered so consecutive
    (block, head) iterations and out-proj chunks pipeline.
  - Scores are small (|s| < ~2), so exp() runs without max-subtraction.
"""

import sys

for _p in ("/opt/trn_rl_repo", "/opt/pypackages"):
    if _p not in sys.path:
        sys.path.insert(0, _p)

import math
import numpy as np
import ml_dtypes

import concourse.bacc as bacc
import concourse.mybir as mybir
import concourse.tile as tile
from concourse.bass_isa import ReduceOp as _ReduceOp
from concourse.bass_utils import run_bass_kernel_spmd

# Problem constants
L = 4096
D = 2048
C = 512
H = 16
HD = 128          # head dim
ROPE = 64
HALF = ROPE // 2  # 32
SPLIT = HD - ROPE # 64
N_CORES = 8
HPC = H // N_CORES   # heads per core = 2
LLOC = L // N_CORES  # 512 (down-proj shard)
BQ = 512             # Lq block
NB = L // BQ         # 8
NKC = L // 128       # 32 Lk chunks
DKC = D // 128       # 16
CKC = C // 128       # 4
ROPE_BASE = 10000.0

BF16 = mybir.dt.bfloat16
F32 = mybir.dt.float32

_CACHE = {}

# Ablation flags for subtractive profiling (timing only; output garbage when set)
OPTS = {
    "no_attn": False,      # skip phase 3 entirely
    "no_outproj": False,   # skip phase 4
    "no_ag": False,        # skip the AllGather collectives
    "no_denom": False,     # skip ones-MMs + recip + broadcast (no normalize)
    "no_upproj": False,    # skip phase 2 (only valid with no_attn+no_outproj)
}


def _build_program(reps=1):
    nc = bacc.Bacc("TRN2", target_bir_lowering=False, debug=False, num_devices=N_CORES)

    xT = nc.dram_tensor("xT", [D, LLOC], BF16, kind="ExternalInput")
    wdkv = nc.dram_tensor("wdkv", [D, C], BF16, kind="ExternalInput")
    wdq = nc.dram_tensor("wdq", [D, C], BF16, kind="ExternalInput")
    wk = nc.dram_tensor("wk", [C, HPC * HD], BF16, kind="ExternalInput")
    wq = nc.dram_tensor("wq", [C, HPC * HD], BF16, kind="ExternalInput")
    wks = nc.dram_tensor("wks", [C, HPC * ROPE], BF16, kind="ExternalInput")
    wqs = nc.dram_tensor("wqs", [C, HPC * ROPE], BF16, kind="ExternalInput")
    wv = nc.dram_tensor("wv", [C, HPC * HD], BF16, kind="ExternalInput")
    wo = nc.dram_tensor("wo", [HPC * HD, D], BF16, kind="ExternalInput")
    CCd = nc.dram_tensor("CC", [128, L], BF16, kind="ExternalInput")
    SSd = nc.dram_tensor("SS", [128, L], BF16, kind="ExternalInput")
    CMd = nc.dram_tensor("CM", [4, 128, BQ], BF16, kind="ExternalInput")
    OUT = nc.dram_tensor("OUT", [L, D], F32, kind="ExternalOutput")

    agin = nc.dram_tensor("agin", [2, C, LLOC], BF16)
    agout = nc.dram_tensor("agout", [N_CORES, 2, C, LLOC], BF16, addr_space="Shared")

    rg = [list(range(N_CORES))]

    with tile.TileContext(nc) as tc:
        for _rep in range(reps):
            _emit_body(nc, tc, locals())
    nc.compile()
    return nc


def _emit_body(nc, tc, g):
    xT, wdkv, wdq = g["xT"], g["wdkv"], g["wdq"]
    wk, wq, wks, wqs, wv, wo = g["wk"], g["wq"], g["wks"], g["wqs"], g["wv"], g["wo"]
    CCd, SSd, CMd, OUT = g["CCd"], g["SSd"], g["CMd"], g["OUT"]
    agin, agout = g["agin"], g["agout"]
    rg = g["rg"]
    Exp = mybir.ActivationFunctionType.Exp
    Copy = mybir.ActivationFunctionType.Copy

    # Single shared PSUM pool, 8 banks:
    #   "s"   [128,2,BQ] x2 bufs = 4 banks   (S^T groups; down-proj psums)
    #   "ctx" [128,BQ]   x2 bufs = 2 banks   (PV accumulate; up-proj mains)
    #   "b1"  [128,BQ]   x1 buf  = 1 bank    (softmax denominator; V psums)
    #   "o"   [128,BQ]   x1 buf  = 1 bank    (out-proj psums; swap-rope psums)
    with tc.tile_pool(name="sb_base", bufs=1) as sbB, tc.tile_pool(
        name="ps", bufs=1, space="PSUM"
    ) as psP:
        kT = sbB.tile([128, HPC, L], BF16, tag="kT")
        qT = sbB.tile([128, HPC, L], BF16, tag="qT")
        vN = sbB.tile([128, NKC, HPC * HD], BF16, tag="vN")
        ctxT = sbB.tile([128, HPC, L], BF16, tag="ctxT")
        wo_t = sbB.tile([128, HPC, D], BF16, tag="wo")
        cm_t = sbB.tile([128, 4, BQ], BF16, tag="cm")

        # ---------------- Phase 1: down-projections + AllGathers ------------
        with tc.tile_pool(name="sb_dp", bufs=1) as sbD:
            xT_t = sbD.tile([128, DKC, LLOC], BF16, tag="xT")
            wdkv_t = sbD.tile([128, DKC, C], BF16, tag="wdkv")
            wdq_t = sbD.tile([128, DKC, C], BF16, tag="wdq")
            xT4 = xT.rearrange("(k p) l -> p k l", p=128)
            wdkv4 = wdkv.rearrange("(k p) c -> p k c", p=128)
            wdq4 = wdq.rearrange("(k p) c -> p k c", p=128)
            for k in range(DKC):
                nc.sync.dma_start(xT_t[:, k, :], xT4[:, k, :])
                nc.sync.dma_start(wdkv_t[:, k, :], wdkv4[:, k, :])
            for k in range(DKC):
                nc.sync.dma_start(wdq_t[:, k, :], wdq4[:, k, :])

            for gi_, (which, w_t) in enumerate((("kv", wdkv_t), ("q", wdq_t))):
                stage = sbD.tile([128, CKC, LLOC], BF16, tag="cstage")
                for t in range(CKC):
                    ps = psP.tile([128, LLOC], F32, tag="s", bufs=2)
                    for k in range(DKC):
                        nc.tensor.matmul(
                            ps[:],
                            w_t[:, k, t * 128 : (t + 1) * 128],
                            xT_t[:, k, :],
                            start=(k == 0),
                            stop=(k == DKC - 1),
                        )
                    nc.vector.tensor_copy(stage[:, t, :], ps[:])
                nc.sync.dma_start(
                    agin[gi_].rearrange("(t p) l -> p t l", p=128), stage[:]
                )
                if gi_ == 1 and not OPTS["no_ag"]:
                    nc.gpsimd.collective_compute(
                        "AllGather",
                        mybir.AluOpType.bypass,
                        replica_groups=rg,
                        ins=[agin[:]],
                        outs=[agout[:]],
                    )


        # ---------------- Phase 2: up-projections + RoPE (per L-window) -----
        with tc.tile_pool(name="sb_up", bufs=1) as sbU:
            wk_t = sbU.tile([128, CKC, HPC * HD], BF16, tag="wk")
            wq_t = sbU.tile([128, CKC, HPC * HD], BF16, tag="wq")
            wks_t = sbU.tile([128, CKC, HPC * ROPE], BF16, tag="wks")
            wqs_t = sbU.tile([128, CKC, HPC * ROPE], BF16, tag="wqs")
            wv_t = sbU.tile([128, CKC, HPC * HD], BF16, tag="wv")
            nc.sync.dma_start(wk_t[:], wk.rearrange("(c p) m -> p c m", p=128))
            nc.sync.dma_start(wq_t[:], wq.rearrange("(c p) m -> p c m", p=128))
            nc.sync.dma_start(wks_t[:], wks.rearrange("(c p) m -> p c m", p=128))
            nc.sync.dma_start(wqs_t[:], wqs.rearrange("(c p) m -> p c m", p=128))
            nc.sync.dma_start(wv_t[:], wv.rearrange("(c p) m -> p c m", p=128))
            cc_t = sbU.tile([128, L], BF16, tag="cc")
            ss_t = sbU.tile([128, L], BF16, tag="ss")
            nc.sync.dma_start(cc_t[:], CCd[:])
            nc.sync.dma_start(ss_t[:], SSd[:])
            nc.sync.dma_start(wo_t[:], wo.rearrange("(h p) d -> p h d", p=128))
            nc.sync.dma_start(cm_t[:], CMd.rearrange("c p l -> p c l"))

            for w in range(NB):
                if OPTS["no_upproj"]:
                    break
                win = slice(w * BQ, (w + 1) * BQ)
                ckw = sbU.tile([128, CKC, BQ], BF16, tag="ckw", bufs=3)
                cqw = sbU.tile([128, CKC, BQ], BF16, tag="cqw", bufs=3)
                nc.sync.dma_start(
                    ckw[:], agout[w, 0].rearrange("(t p) l -> p t l", p=128)
                )
                nc.sync.dma_start(
                    cqw[:], agout[w, 1].rearrange("(t p) l -> p t l", p=128)
                )

                for dst, w_t, ws_t, src in (
                    (kT, wk_t, wks_t, ckw),
                    (qT, wq_t, wqs_t, cqw),
                ):
                    # swapped-rope projection, both heads in one psum:
                    # rows 0:64 = head0 swap (head0 rope rows), 64:128 = head1
                    ps_sw = psP.tile([128, BQ], F32, tag="o", bufs=2)
                    for c in range(CKC):
                        nc.tensor.matmul(
                            ps_sw[:],
                            ws_t[:, c, :],
                            src[:, c, :],
                            start=(c == 0),
                            stop=(c == CKC - 1),
                        )
                    swm = sbU.tile([128, BQ], BF16, tag="swm", bufs=2)
                    nc.vector.tensor_mul(swm[:], ps_sw[:], ss_t[:, win])

                    for h in range(HPC):
                        ps = psP.tile([128, BQ], F32, tag="ctx", bufs=2)
                        for c in range(CKC):
                            nc.tensor.matmul(
                                ps[:],
                                w_t[:, c, h * HD : (h + 1) * HD],
                                src[:, c, :],
                                start=(c == 0),
                                stop=(c == CKC - 1),
                            )
                        # head0 layout [rope;base]: rope rows 0:64
                        # head1 layout [base;rope]: rope rows 64:128
                        ro = slice(0, 64) if h == 0 else slice(64, 128)
                        ba = slice(64, 128) if h == 0 else slice(0, 64)
                        nc.scalar.activation(dst[ba, h, win], ps[ba, :], Copy)
                        nc.vector.tensor_mul(
                            dst[ro, h, win], ps[ro, :], cc_t[ro, win]
                        )
                        nc.vector.tensor_add(
                            dst[ro, h, win], dst[ro, h, win], swm[ro, :]
                        )

                # V: natural layout [Lk, d] chunks (both heads, N=256)
                for j in range(4):
                    lc = w * 4 + j
                    ps = psP.tile([128, BQ], F32, tag="o", bufs=2)
                    for c in range(CKC):
                        nc.tensor.matmul(
                            ps[:, 0 : HPC * HD],
                            ckw[:, c, j * 128 : (j + 1) * 128],
                            wv_t[:, c, :],
                            start=(c == 0),
                            stop=(c == CKC - 1),
                        )
                    nc.vector.tensor_copy(vN[:, lc, :], ps[:, 0 : HPC * HD])

        # -------- Phase 3: attention + fused out-projection (per q-block) ---
        with tc.tile_pool(name="sb_at", bufs=1) as sbA:
            # b=0 first (ready earliest), then largest blocks, smallest last
            # so the pipeline tail is short
            for b in (0, 7, 6, 5, 4, 3, 2, 1):
                if OPTS["no_attn"]:
                    break
                nch = 4 * (b + 1)
                qwin = slice(b * BQ, (b + 1) * BQ)
                for h in range(HPC):
                    expS = sbA.tile([128, NKC, BQ], BF16, tag="expS", bufs=2)
                    acc = sbA.tile([128, BQ], F32, tag="acc", bufs=2)
                    ctx_ps = psP.tile([128, BQ], F32, tag="ctx", bufs=2)
                    epair = None
                    # zero the dead prefixes of the 3 upper diagonal chunks so
                    # the denominator tree can read full-width rows
                    for dj in range(1, 4):
                        nc.vector.memset(expS[:, nch - 4 + dj, 0 : 128 * dj], 0.0)
                    ck0 = 0
                    while ck0 < nch:
                        gsz = min(2, nch - ck0)
                        s_ps = psP.tile([128, 2, BQ], F32, tag="s", bufs=2)
                        for j in range(gsz):
                            ck = ck0 + j
                            cs = 128 * (ck - (nch - 4)) if ck >= nch - 4 else 0
                            nc.tensor.matmul(
                                s_ps[:, j, cs:],
                                kT[:, h, ck * 128 : (ck + 1) * 128],
                                qT[:, h, b * BQ + cs : (b + 1) * BQ],
                                start=True,
                                stop=True,
                            )
                        cs0 = 128 * (ck0 - (nch - 4)) if ck0 >= nch - 4 else 0
                        if cs0 == 0 and (ck0 + gsz <= nch - 4 or ck0 == nch - 4):
                            # both chunks full-width (or first diag pair j0):
                            # j0 is full, j1 needs its own range
                            if ck0 + gsz <= nch - 4:
                                nc.scalar.activation(
                                    expS[:, ck0 : ck0 + gsz, :], s_ps[:, 0:gsz, :], Exp
                                )
                            else:
                                nc.scalar.activation(
                                    expS[:, ck0, :], s_ps[:, 0, :], Exp
                                )
                                nc.scalar.activation(
                                    expS[:, ck0 + 1, 128:], s_ps[:, 1, 128:], Exp
                                )
                        else:
                            for j in range(gsz):
                                ck = ck0 + j
                                cs = 128 * (ck - (nch - 4)) if ck >= nch - 4 else 0
                                nc.scalar.activation(
                                    expS[:, ck, cs:], s_ps[:, j, cs:], Exp
                                )
                        for j in range(gsz):
                            ck = ck0 + j
                            if ck >= nch - 4:  # diagonal chunk: causal mask
                                cs = 128 * (ck - (nch - 4))
                                nc.vector.tensor_mul(
                                    expS[:, ck, cs:],
                                    expS[:, ck, cs:],
                                    cm_t[:, ck - (nch - 4), cs:],
                                )
                        if not OPTS["no_denom"]:
                            # bf16 pair/quad tree + f32 quad accumulate, all
                            # on DVE (no PE cycles for the denominator)
                            if ck0 % 4 == 0:
                                epair = sbA.tile([128, BQ], BF16, tag="epair", bufs=2)
                                nc.vector.tensor_add(
                                    epair[:], expS[:, ck0, :], expS[:, ck0 + 1, :]
                                )
                            else:
                                equad = sbA.tile([128, BQ], BF16, tag="equad", bufs=2)
                                nc.vector.tensor_add(
                                    equad[:], expS[:, ck0, :], expS[:, ck0 + 1, :]
                                )
                                nc.vector.tensor_add(equad[:], equad[:], epair[:])
                                if ck0 == 2:
                                    nc.vector.tensor_copy(acc[:], equad[:])
                                else:
                                    nc.vector.tensor_add(acc[:], acc[:], equad[:])
                        for j in range(gsz):
                            ck = ck0 + j
                            cs = 128 * (ck - (nch - 4)) if ck >= nch - 4 else 0
                            nc.tensor.matmul(
                                ctx_ps[:, cs:],
                                vN[:, ck, h * HD : (h + 1) * HD],
                                expS[:, ck, cs:],
                                start=(ck == 0),
                                stop=(ck == nch - 1),
                                skip_group_check=(cs > 0),
                            )
                        ck0 += gsz
                    if OPTS["no_denom"]:
                        nc.vector.tensor_copy(ctxT[:, h, qwin], ctx_ps[:])
                        continue
                    dsum_t = sbA.tile([128, BQ], F32, tag="dsum_t", bufs=2)
                    nc.gpsimd.partition_all_reduce(dsum_t[:], acc[:], channels=128, reduce_op=_ReduceOp.add)
                    bc_t = sbA.tile([128, BQ], F32, tag="bc_t", bufs=2)
                    nc.vector.reciprocal_approx_fast(out=bc_t[:], in_=dsum_t[:])
                    nc.vector.tensor_mul(ctxT[:, h, qwin], ctx_ps[:], bc_t[:])

                # fused out-projection for this q-window (4 row-chunks of 128)
                if OPTS["no_outproj"]:
                    continue
                for j in range(4):
                    lc = b * 4 + j
                    ostage = sbA.tile([128, D], F32, tag="ostage", bufs=3)
                    for do in range(4):
                        ps = psP.tile([128, BQ], F32, tag="o", bufs=2)
                        for h in range(HPC):
                            nc.tensor.matmul(
                                ps[:],
                                ctxT[:, h, lc * 128 : (lc + 1) * 128],
                                wo_t[:, h, do * 512 : (do + 1) * 512],
                                start=(h == 0),
                                stop=(h == HPC - 1),
                            )
                        # psum drain copies alternate DVE / ACT
                        if do % 2 == 0:
                            nc.vector.tensor_copy(
                                ostage[:, do * 512 : (do + 1) * 512], ps[:]
                            )
                        else:
                            nc.scalar.activation(
                                ostage[:, do * 512 : (do + 1) * 512], ps[:], Copy
                            )
                        if do == 1:
                            nc.sync.dma_start(
                                OUT[lc * 128 : (lc + 1) * 128, 0:1024],
                                ostage[:, 0:1024],
                            )
                    nc.sync.dma_start(
                        OUT[lc * 128 : (lc + 1) * 128, 1024:2048],
                        ostage[:, 1024:2048],
                    )


def _host_inputs(x, W_dkv, W_dq, W_uk, W_uv, W_uq, W_qr, W_kr, W_out):
    """Build per-core input maps (numpy, bf16)."""
    bf = lambda a: np.ascontiguousarray(a).astype(ml_dtypes.bfloat16)
    scale = 1.0 / math.sqrt(HD)

    xt = np.ascontiguousarray(x.reshape(L, D).T)  # [D, L] f32

    # rope tables, transposed: ang[l, i] = l * inv_freq[i]
    inv_freq = 1.0 / (ROPE_BASE ** (np.arange(HALF, dtype=np.float64) * 2.0 / ROPE))
    ang = np.arange(L, dtype=np.float64)[:, None] * inv_freq[None, :]  # [L, 32]
    cosT = np.cos(ang).T.astype(np.float32)  # [32, L]
    sinT = np.sin(ang).T.astype(np.float32)
    cc64 = np.concatenate([cosT, cosT], axis=0)        # [64, L]
    ss64 = np.concatenate([-sinT, sinT], axis=0)       # [64, L]
    CC = np.concatenate([cc64, cc64], axis=0)          # [128, L] (both head slots)
    SS = np.concatenate([ss64, ss64], axis=0)          # [128, L]

    CM = np.zeros((4, 128, BQ), dtype=np.float32)
    for j in range(4):
        for lk in range(128):
            CM[j, lk, j * 128 + lk :] = 1.0

    in_maps = []
    for r in range(N_CORES):
        heads = [2 * r, 2 * r + 1]
        wk_blocks, wq_blocks, wks_blocks, wqs_blocks, wv_blocks = [], [], [], [], []
        for i, h in enumerate(heads):
            kr = W_kr[:, h * ROPE : (h + 1) * ROPE]
            kb = W_uk[:, h * SPLIT : (h + 1) * SPLIT]
            qr = W_qr[:, h * ROPE : (h + 1) * ROPE] * scale
            qb = W_uq[:, h * SPLIT : (h + 1) * SPLIT] * scale
            # head-dim order: slot0 head [rope;base], slot1 head [base;rope]
            # (rope rows land at partitions 0:64 / 64:128 respectively)
            if i == 0:
                wk_blocks.append(np.concatenate([kr, kb], axis=1))
                wq_blocks.append(np.concatenate([qr, qb], axis=1))
            else:
                wk_blocks.append(np.concatenate([kb, kr], axis=1))
                wq_blocks.append(np.concatenate([qb, qr], axis=1))
            # swapped rope halves for the rotate-half term (64 cols per head)
            wks_blocks.append(np.concatenate([kr[:, HALF:], kr[:, :HALF]], axis=1))
            wqs_blocks.append(np.concatenate([qr[:, HALF:], qr[:, :HALF]], axis=1))
            wv_blocks.append(W_uv[:, h * HD : (h + 1) * HD])
        wo_rows = np.concatenate(
            [W_out[h * HD : (h + 1) * HD, :] for h in heads], axis=0
        )
        in_maps.append(
            {
                "xT": bf(xt[:, r * LLOC : (r + 1) * LLOC]),
                "wdkv": bf(W_dkv),
                "wdq": bf(W_dq),
                "wk": bf(np.concatenate(wk_blocks, axis=1)),
                "wq": bf(np.concatenate(wq_blocks, axis=1)),
                "wks": bf(np.concatenate(wks_blocks, axis=1)),
                "wqs": bf(np.concatenate(wqs_blocks, axis=1)),
                "wv": bf(np.concatenate(wv_blocks, axis=1)),
                "wo": bf(wo_rows),
                "CC": bf(CC),
                "SS": bf(SS),
                "CM": bf(CM),
            }
        )
    return in_maps


def _get_program(reps=1):
    if reps not in _CACHE:
        _CACHE[reps] = _build_program(reps)
    return _CACHE[reps]


def make_runner(in_maps, reps=1, empty=False):
    """Persistent compiled runner for timing: returns run_chain(M) that executes
    the program M times back-to-back on device (chained via the output buffer so
    executions serialize), returning wall seconds for the chain."""
    import time as _time
    import jax
    from jax.sharding import Mesh, PartitionSpec, NamedSharding
    from jax.experimental.shard_map import shard_map
    import concourse.bass2jax as bass2jax

    nc = _EMPTY_CACHE.setdefault(0, _build_empty()) if empty else _get_program(reps)
    bass2jax.install_neuronx_cc_hook()
    partition_name = nc.partition_id_tensor.name if nc.partition_id_tensor else None
    in_names, out_names, out_avals, zero_outs = [], [], [], []
    for alloc in nc.m.functions[0].allocations:
        if not isinstance(alloc, mybir.MemoryLocationSet):
            continue
        name = alloc.memorylocations[0].name
        if alloc.kind == "ExternalInput":
            if name != partition_name:
                in_names.append(name)
        elif alloc.kind == "ExternalOutput":
            out_names.append(name)
            shape = tuple(alloc.tensor_shape)
            dtype = mybir.dt.np(alloc.dtype)
            out_avals.append(jax.core.ShapedArray(shape, dtype))
            zero_outs.append(np.zeros(shape, dtype))
    n_params = len(in_names)
    in_names_all = in_names + out_names
    if partition_name is not None:
        in_names_all = in_names_all + [partition_name]

    def _body(*args):
        operands = list(args)
        if partition_name is not None:
            operands.append(bass2jax.partition_id_tensor())
        outs = bass2jax._bass_exec_p.bind(
            *operands,
            out_avals=tuple(out_avals),
            in_names=tuple(in_names_all),
            out_names=tuple(out_names),
            lowering_input_output_aliases=(),
            sim_require_finite=True,
            sim_require_nnan=True,
            nc=nc,
        )
        return tuple(outs)

    devices = jax.devices()[:N_CORES]
    mesh = Mesh(np.asarray(devices), ("core",))
    n_outs = len(out_names)
    in_specs = (PartitionSpec("core"),) * (n_params + n_outs)
    out_specs = (PartitionSpec("core"),) * n_outs
    sharded = jax.jit(
        shard_map(_body, mesh=mesh, in_specs=in_specs, out_specs=out_specs, check_rep=False),
        keep_unused=True,
    )
    sh = NamedSharding(mesh, PartitionSpec("core"))
    concat_in = [
        np.concatenate([np.asarray(in_maps[c][nm]) for c in range(N_CORES)], axis=0)
        for nm in in_names
    ]
    concat_zeros = [
        np.zeros((N_CORES * z.shape[0], *z.shape[1:]), z.dtype) for z in zero_outs
    ]
    dev_in = [jax.device_put(a, sh) for a in concat_in]
    dev_zero = [jax.device_put(a, sh) for a in concat_zeros]
    outs = sharded(*dev_in, *dev_zero)
    jax.block_until_ready(outs)  # compile + warm

    def run_chain(M):
        z = list(dev_zero)
        t0 = _time.perf_counter()
        outs = None
        for _ in range(M):
            outs = sharded(*dev_in, *z)
            z = list(outs)
        jax.block_until_ready(outs)
        return _time.perf_counter() - t0

    return run_chain


_EMPTY_CACHE = {}


def _build_empty():
    """Minimal program with the same I/O signature class: one tiny DMA."""
    nc = bacc.Bacc("TRN2", target_bir_lowering=False, debug=False, num_devices=N_CORES)
    xT = nc.dram_tensor("xT", [128, 128], F32, kind="ExternalInput")
    OUT = nc.dram_tensor("OUT", [128, 128], F32, kind="ExternalOutput")
    with tile.TileContext(nc) as tc:
        with tc.tile_pool(name="sb", bufs=1) as sb:
            t = sb.tile([128, 128], F32, tag="t")
            nc.sync.dma_start(t[:], xT[:])
            nc.sync.dma_start(OUT[:], t[:])
    nc.compile()
    return nc


def make_empty_runner():
    in_maps = [{"xT": np.zeros((128, 128), np.float32)} for _ in range(N_CORES)]
    return make_runner(in_maps, empty=True)


def kernel(x, W_dkv, W_dq, W_uk, W_uv, W_uq, W_qr, W_kr, W_out, b_out, reps=1):
    x = np.asarray(x, dtype=np.float32)
    in_maps = _host_inputs(
        x,
        np.asarray(W_dkv, np.float32), np.asarray(W_dq, np.float32),
        np.asarray(W_uk, np.float32), np.asarray(W_uv, np.float32),
        np.asarray(W_uq, np.float32), np.asarray(W_qr, np.float32),
        np.asarray(W_kr, np.float32), np.asarray(W_out, np.float32),
    )
    nc = _get_program(reps)
    res = run_bass_kernel_spmd(nc, in_maps, core_ids=list(range(N_CORES)), trace=False)
    out = np.zeros((L, D), dtype=np.float32)
    for r in range(N_CORES):
        out += res.results[r]["OUT"]
    out += np.asarray(b_out, np.float32)[None, :]
    return out.reshape(1, L, D)



# revision 33
# speedup vs baseline: 1.1276x; 1.0614x over previous
"""Multi-Head Latent Attention (MLA) forward on 8 Trainium2 NeuronCores.

Contract: kernel(**inputs) takes the FULL unsharded inputs (numpy) and
returns the FULL [1, 4096, 2048] float32 output.

Sharding (hardcoded):
  - Tensor-parallel over heads: 2 heads per core (W_uk/W_uv/W_uq/W_qr/W_kr
    column-sharded, W_out row-sharded; partial outputs summed on host).
  - Down-projections (x @ W_dkv, x @ W_dq) are sharded over the sequence
    (512 rows per core) and AllGathered in bf16.

Device layout notes:
  - All activations are kept transposed ([dim, L]) so every matmul in the
    chain uses natural-layout weights and never needs an on-device transpose.
  - q/k head dims are reordered to [rope(64); base(64)] so the RoPE rows sit
    at partitions 0..63; the RoPE half-swap (x1,x2 -> x2,x1) is folded into
    extra "swapped" weight columns, so RoPE is pure lane-aligned elementwise.
  - Attention runs in the S^T = [Lk, Lq] orientation: softmax denominators
    are summed chunk-wise with a bf16 pair/quad tree + f32 accumulator on the
    DVE (no PE cycles), then reduced across partitions with one
    gpsimd.partition_all_reduce per (block, head).
  - Score and PV matmuls for the causally-dead column ranges of the 4
    diagonal chunks are skipped (the masked prefix of expS is memset to 0).
  - expS and the out-proj psum are double-buffered so consecutive
    (block, head) iterations and out-proj chunks pipeline.
  - Scores are small (|s| < ~2), so exp() runs without max-subtraction.
"""

import sys

for _p in ("/opt/trn_rl_repo", "/opt/pypackages"):
    if _p not in sys.path:
        sys.path.insert(0, _p)

import math
import numpy as np
import ml_dtypes

import concourse.bacc as bacc
import concourse.mybir as mybir
import concourse.tile as tile
from concourse.bass_isa import ReduceOp as _ReduceOp
from concourse.bass_utils import run_bass_kernel_spmd

# Problem constants
L = 4096
D = 2048
C = 512
H = 16
HD = 128          # head dim
ROPE = 64
HALF = ROPE // 2  # 32
SPLIT = HD - ROPE # 64
N_CORES = 8
HPC = H // N_CORES   # heads per core = 2
LLOC = L // N_CORES  # 512 (down-proj shard)
BQ = 512             # Lq block
NB = L // BQ         # 8
NKC = L // 128       # 32 Lk chunks
DKC = D // 128       # 16
CKC = C // 128       # 4
ROPE_BASE = 10000.0

BF16 = mybir.dt.bfloat16
F32 = mybir.dt.float32

_CACHE = {}

# Ablation flags for subtractive profiling (timing only; output garbage when set)
OPTS = {
    "no_attn": False,      # skip phase 3 entirely
    "no_outproj": False,   # skip phase 4
    "no_ag": False,        # skip the AllGather collectives
    "no_denom": False,     # skip ones-MMs + recip + broadcast (no normalize)
    "no_upproj": False,    # skip phase 2 (only valid with no_attn+no_outproj)
}


def _build_program(reps=1):
    nc = bacc.Bacc("TRN2", target_bir_lowering=False, debug=False, num_devices=N_CORES)

    xT = nc.dram_tensor("xT", [D, LLOC], BF16, kind="ExternalInput")
    wdkv = nc.dram_tensor("wdkv", [D, C], BF16, kind="ExternalInput")
    wdq = nc.dram_tensor("wdq", [D, C], BF16, kind="ExternalInput")
    wk = nc.dram_tensor("wk", [C, HPC * HD], BF16, kind="ExternalInput")
    wq = nc.dram_tensor("wq", [C, HPC * HD], BF16, kind="ExternalInput")
    wks = nc.dram_tensor("wks", [C, HPC * ROPE], BF16, kind="ExternalInput")
    wqs = nc.dram_tensor("wqs", [C, HPC * ROPE], BF16, kind="ExternalInput")
    wv = nc.dram_tensor("wv", [C, HPC * HD], BF16, kind="ExternalInput")
    wo = nc.dram_tensor("wo", [HPC * HD, D], BF16, kind="ExternalInput")
    CCd = nc.dram_tensor("CC", [128, L], BF16, kind="ExternalInput")
    SSd = nc.dram_tensor("SS", [128, L], BF16, kind="ExternalInput")
    CMd = nc.dram_tensor("CM", [4, 128, BQ], BF16, kind="ExternalInput")
    OUT = nc.dram_tensor("OUT", [L, D], F32, kind="ExternalOutput")

    agin = nc.dram_tensor("agin", [2, C, LLOC], BF16)
    agout = nc.dram_tensor("agout", [N_CORES, 2, C, LLOC], BF16, addr_space="Shared")

    rg = [list(range(N_CORES))]

    with tile.TileContext(nc) as tc:
        for _rep in range(reps):
            _emit_body(nc, tc, locals())
    nc.compile()
    return nc


def _emit_body(nc, tc, g):
    xT, wdkv, wdq = g["xT"], g["wdkv"], g["wdq"]
    wk, wq, wks, wqs, wv, wo = g["wk"], g["wq"], g["wks"], g["wqs"], g["wv"], g["wo"]
    CCd, SSd, CMd, OUT = g["CCd"], g["SSd"], g["CMd"], g["OUT"]
    agin, agout = g["agin"], g["agout"]
    rg = g["rg"]
    Exp = mybir.ActivationFunctionType.Exp
    Copy = mybir.ActivationFunctionType.Copy

    # Single shared PSUM pool, 8 banks:
    #   "s"   [128,2,BQ] x2 bufs = 4 banks   (S^T groups; down-proj psums)
    #   "ctx" [128,BQ]   x2 bufs = 2 banks   (PV accumulate; up-proj mains)
    #   "b1"  [128,BQ]   x1 buf  = 1 bank    (softmax denominator; V psums)
    #   "o"   [128,BQ]   x1 buf  = 1 bank    (out-proj psums; swap-rope psums)
    with tc.tile_pool(name="sb_base", bufs=1) as sbB, tc.tile_pool(
        name="ps", bufs=1, space="PSUM"
    ) as psP:
        kT = sbB.tile([128, HPC, L], BF16, tag="kT")
        qT = sbB.tile([128, HPC, L], BF16, tag="qT")
        vN = sbB.tile([128, NKC, HPC * HD], BF16, tag="vN")
        ctxT = sbB.tile([128, HPC, L], BF16, tag="ctxT")
        wo_t = sbB.tile([128, HPC, D], BF16, tag="wo")
        cm_t = sbB.tile([128, 4, BQ], BF16, tag="cm")

        # ---------------- Phase 1: down-projections + AllGathers ------------
        with tc.tile_pool(name="sb_dp", bufs=1) as sbD:
            xT_t = sbD.tile([128, DKC, LLOC], BF16, tag="xT")
            wdkv_t = sbD.tile([128, DKC, C], BF16, tag="wdkv")
            wdq_t = sbD.tile([128, DKC, C], BF16, tag="wdq")
            xT4 = xT.rearrange("(k p) l -> p k l", p=128)
            wdkv4 = wdkv.rearrange("(k p) c -> p k c", p=128)
            wdq4 = wdq.rearrange("(k p) c -> p k c", p=128)
            for k in range(DKC):
                nc.sync.dma_start(xT_t[:, k, :], xT4[:, k, :])
                nc.sync.dma_start(wdkv_t[:, k, :], wdkv4[:, k, :])
            for k in range(DKC):
                nc.sync.dma_start(wdq_t[:, k, :], wdq4[:, k, :])

            for gi_, (which, w_t) in enumerate((("kv", wdkv_t), ("q", wdq_t))):
                stage = sbD.tile([128, CKC, LLOC], BF16, tag="cstage")
                for t in range(CKC):
                    ps = psP.tile([128, LLOC], F32, tag="s", bufs=2)
                    for k in range(DKC):
                        nc.tensor.matmul(
                            ps[:],
                            w_t[:, k, t * 128 : (t + 1) * 128],
                            xT_t[:, k, :],
                            start=(k == 0),
                            stop=(k == DKC - 1),
                        )
                    nc.vector.tensor_copy(stage[:, t, :], ps[:])
                nc.sync.dma_start(
                    agin[gi_].rearrange("(t p) l -> p t l", p=128), stage[:]
                )
                if gi_ == 1 and not OPTS["no_ag"]:
                    nc.gpsimd.collective_compute(
                        "AllGather",
                        mybir.AluOpType.bypass,
                        replica_groups=rg,
                        ins=[agin[:]],
                        outs=[agout[:]],
                    )


        # ---------------- Phase 2: up-projections + RoPE (per L-window) -----
        with tc.tile_pool(name="sb_up", bufs=1) as sbU:
            wk_t = sbU.tile([128, CKC, HPC * HD], BF16, tag="wk")
            wq_t = sbU.tile([128, CKC, HPC * HD], BF16, tag="wq")
            wks_t = sbU.tile([128, CKC, HPC * ROPE], BF16, tag="wks")
            wqs_t = sbU.tile([128, CKC, HPC * ROPE], BF16, tag="wqs")
            wv_t = sbU.tile([128, CKC, HPC * HD], BF16, tag="wv")
            nc.sync.dma_start(wk_t[:], wk.rearrange("(c p) m -> p c m", p=128))
            nc.sync.dma_start(wq_t[:], wq.rearrange("(c p) m -> p c m", p=128))
            nc.sync.dma_start(wks_t[:], wks.rearrange("(c p) m -> p c m", p=128))
            nc.sync.dma_start(wqs_t[:], wqs.rearrange("(c p) m -> p c m", p=128))
            nc.sync.dma_start(wv_t[:], wv.rearrange("(c p) m -> p c m", p=128))
            cc_t = sbU.tile([128, L], BF16, tag="cc")
            ss_t = sbU.tile([128, L], BF16, tag="ss")
            nc.sync.dma_start(cc_t[:], CCd[:])
            nc.sync.dma_start(ss_t[:], SSd[:])
            nc.sync.dma_start(wo_t[:], wo.rearrange("(h p) d -> p h d", p=128))
            nc.sync.dma_start(cm_t[:], CMd.rearrange("c p l -> p c l"))

            for w in range(NB):
                if OPTS["no_upproj"]:
                    break
                win = slice(w * BQ, (w + 1) * BQ)
                ckw = sbU.tile([128, CKC, BQ], BF16, tag="ckw", bufs=3)
                cqw = sbU.tile([128, CKC, BQ], BF16, tag="cqw", bufs=3)
                nc.sync.dma_start(
                    ckw[:], agout[w, 0].rearrange("(t p) l -> p t l", p=128)
                )
                nc.sync.dma_start(
                    cqw[:], agout[w, 1].rearrange("(t p) l -> p t l", p=128)
                )

                for dst, w_t, ws_t, src in (
                    (kT, wk_t, wks_t, ckw),
                    (qT, wq_t, wqs_t, cqw),
                ):
                    # swapped-rope projection, both heads in one psum:
                    # rows 0:64 = head0 swap (head0 rope rows), 64:128 = head1
                    ps_sw = psP.tile([128, BQ], F32, tag="o", bufs=2)
                    for c in range(CKC):
                        nc.tensor.matmul(
                            ps_sw[:],
                            ws_t[:, c, :],
                            src[:, c, :],
                            start=(c == 0),
                            stop=(c == CKC - 1),
                        )
                    swm = sbU.tile([128, BQ], BF16, tag="swm", bufs=2)
                    nc.vector.tensor_mul(swm[:], ps_sw[:], ss_t[:, win])

                    for h in range(HPC):
                        ps = psP.tile([128, BQ], F32, tag="ctx", bufs=2)
                        for c in range(CKC):
                            nc.tensor.matmul(
                                ps[:],
                                w_t[:, c, h * HD : (h + 1) * HD],
                                src[:, c, :],
                                start=(c == 0),
                                stop=(c == CKC - 1),
                            )
                        # head0 layout [rope;base]: rope rows 0:64
                        # head1 layout [base;rope]: rope rows 64:128
                        ro = slice(0, 64) if h == 0 else slice(64, 128)
                        ba = slice(64, 128) if h == 0 else slice(0, 64)
                        nc.scalar.activation(dst[ba, h, win], ps[ba, :], Copy)
                        nc.vector.tensor_mul(
                            dst[ro, h, win], ps[ro, :], cc_t[ro, win]
                        )
                        nc.vector.tensor_add(
                            dst[ro, h, win], dst[ro, h, win], swm[ro, :]
                        )

                # V: natural layout [Lk, d] chunks (both heads, N=256)
                for j in range(4):
                    lc = w * 4 + j
                    ps = psP.tile([128, BQ], F32, tag="o", bufs=2)
                    for c in range(CKC):
                        nc.tensor.matmul(
                            ps[:, 0 : HPC * HD],
                            ckw[:, c, j * 128 : (j + 1) * 128],
                            wv_t[:, c, :],
                            start=(c == 0),
                            stop=(c == CKC - 1),
                        )
                    nc.vector.tensor_copy(vN[:, lc, :], ps[:, 0 : HPC * HD])

        # -------- Phase 3: attention + fused out-projection (per q-block) ---
        with tc.tile_pool(name="sb_at", bufs=1) as sbA:
            # b=0 first (ready earliest), then largest blocks, smallest last
            # so the pipeline tail is short
            for b in (0, 7, 6, 5, 4, 3, 2, 1):
                if OPTS["no_attn"]:
                    break
                nch = 4 * (b + 1)
                qwin = slice(b * BQ, (b + 1) * BQ)
                for h in range(HPC):
                    expS = sbA.tile([128, NKC, BQ], BF16, tag="expS", bufs=2)
                    acc = sbA.tile([128, BQ], F32, tag="acc", bufs=2)
                    ctx_ps = psP.tile([128, BQ], F32, tag="ctx", bufs=2)
                    epair = None
                    # zero the dead prefixes of the 3 upper diagonal chunks so
                    # the denominator tree can read full-width rows
                    for dj in range(1, 4):
                        nc.vector.memset(expS[:, nch - 4 + dj, 0 : 128 * dj], 0.0)
                    ck0 = 0
                    while ck0 < nch:
                        gsz = min(2, nch - ck0)
                        s_ps = psP.tile([128, 2, BQ], F32, tag="s", bufs=2)
                        for j in range(gsz):
                            ck = ck0 + j
                            cs = 128 * (ck - (nch - 4)) if ck >= nch - 4 else 0
                            nc.tensor.matmul(
                                s_ps[:, j, cs:],
                                kT[:, h, ck * 128 : (ck + 1) * 128],
                                qT[:, h, b * BQ + cs : (b + 1) * BQ],
                                start=True,
                                stop=True,
                            )
                        cs0 = 128 * (ck0 - (nch - 4)) if ck0 >= nch - 4 else 0
                        if cs0 == 0 and (ck0 + gsz <= nch - 4 or ck0 == nch - 4):
                            # both chunks full-width (or first diag pair j0):
                            # j0 is full, j1 needs its own range
                            if ck0 + gsz <= nch - 4:
                                nc.scalar.activation(
                                    expS[:, ck0 : ck0 + gsz, :], s_ps[:, 0:gsz, :], Exp
                                )
                            else:
                                nc.scalar.activation(
                                    expS[:, ck0, :], s_ps[:, 0, :], Exp
                                )
                                nc.scalar.activation(
                                    expS[:, ck0 + 1, 128:], s_ps[:, 1, 128:], Exp
                                )
                        else:
                            for j in range(gsz):
                                ck = ck0 + j
                                cs = 128 * (ck - (nch - 4)) if ck >= nch - 4 else 0
                                nc.scalar.activation(
                                    expS[:, ck, cs:], s_ps[:, j, cs:], Exp
                                )
                        for j in range(gsz):
                            ck = ck0 + j
                            if ck >= nch - 4:  # diagonal chunk: causal mask
                                cs = 128 * (ck - (nch - 4))
                                nc.vector.tensor_mul(
                                    expS[:, ck, cs:],
                                    expS[:, ck, cs:],
                                    cm_t[:, ck - (nch - 4), cs:],
                                )
                        if not OPTS["no_denom"]:
                            # bf16 pair/quad/oct tree on DVE (cheap 16-bit
                            # adds), f32 accumulate once per oct (or leftover
                            # quad). No PE cycles for the denominator.
                            if ck0 % 4 == 0:
                                epair = sbA.tile([128, BQ], BF16, tag="epair", bufs=2)
                                nc.vector.tensor_add(
                                    epair[:], expS[:, ck0, :], expS[:, ck0 + 1, :]
                                )
                            else:
                                equad = sbA.tile([128, BQ], BF16, tag="equad", bufs=2)
                                nc.vector.tensor_add(
                                    equad[:], expS[:, ck0, :], expS[:, ck0 + 1, :]
                                )
                                nc.vector.tensor_add(equad[:], equad[:], epair[:])
                                qi = ck0 // 4  # quad index
                                if qi % 2 == 0:
                                    if qi == nch // 4 - 1:
                                        # leftover quad (odd quad count)
                                        if qi == 0:
                                            nc.vector.tensor_copy(acc[:], equad[:])
                                        else:
                                            nc.vector.tensor_add(
                                                acc[:], acc[:], equad[:]
                                            )
                                    else:
                                        pquad = equad  # defer to oct partner
                                else:
                                    eoct = sbA.tile([128, BQ], BF16, tag="eoct", bufs=2)
                                    nc.vector.tensor_add(
                                        eoct[:], pquad[:], equad[:]
                                    )
                                    if qi == 1:
                                        nc.vector.tensor_copy(acc[:], eoct[:])
                                    else:
                                        nc.vector.tensor_add(acc[:], acc[:], eoct[:])
                        for j in range(gsz):
                            ck = ck0 + j
                            cs = 128 * (ck - (nch - 4)) if ck >= nch - 4 else 0
                            nc.tensor.matmul(
                                ctx_ps[:, cs:],
                                vN[:, ck, h * HD : (h + 1) * HD],
                                expS[:, ck, cs:],
                                start=(ck == 0),
                                stop=(ck == nch - 1),
                                skip_group_check=(cs > 0),
                            )
                        ck0 += gsz
                    if OPTS["no_denom"]:
                        nc.vector.tensor_copy(ctxT[:, h, qwin], ctx_ps[:])
                        continue
                    dsum_t = sbA.tile([128, BQ], F32, tag="dsum_t", bufs=2)
                    nc.gpsimd.partition_all_reduce(dsum_t[:], acc[:], channels=128, reduce_op=_ReduceOp.add)
                    bc_t = sbA.tile([128, BQ], F32, tag="bc_t", bufs=2)
                    nc.vector.reciprocal_approx_fast(out=bc_t[:], in_=dsum_t[:])
                    nc.vector.tensor_mul(ctxT[:, h, qwin], ctx_ps[:], bc_t[:])

                # fused out-projection for this q-window (4 row-chunks of 128)
                if OPTS["no_outproj"]:
                    continue
                for j in range(4):
                    lc = b * 4 + j
                    ostage = sbA.tile([128, D], F32, tag="ostage", bufs=3)
                    for do in range(4):
                        ps = psP.tile([128, BQ], F32, tag="o", bufs=2)
                        for h in range(HPC):
                            nc.tensor.matmul(
                                ps[:],
                                ctxT[:, h, lc * 128 : (lc + 1) * 128],
                                wo_t[:, h, do * 512 : (do + 1) * 512],
                                start=(h == 0),
                                stop=(h == HPC - 1),
                            )
                        # psum drain copies alternate DVE / ACT
                        if do % 2 == 0:
                            nc.vector.tensor_copy(
                                ostage[:, do * 512 : (do + 1) * 512], ps[:]
                            )
                        else:
                            nc.scalar.activation(
                                ostage[:, do * 512 : (do + 1) * 512], ps[:], Copy
                            )
                        if do == 1:
                            nc.sync.dma_start(
                                OUT[lc * 128 : (lc + 1) * 128, 0:1024],
                                ostage[:, 0:1024],
                            )
                    nc.sync.dma_start(
                        OUT[lc * 128 : (lc + 1) * 128, 1024:2048],
                        ostage[:, 1024:2048],
                    )


def _host_inputs(x, W_dkv, W_dq, W_uk, W_uv, W_uq, W_qr, W_kr, W_out):
    """Build per-core input maps (numpy, bf16)."""
    bf = lambda a: np.ascontiguousarray(a).astype(ml_dtypes.bfloat16)
    scale = 1.0 / math.sqrt(HD)

    xt = np.ascontiguousarray(x.reshape(L, D).T)  # [D, L] f32

    # rope tables, transposed: ang[l, i] = l * inv_freq[i]
    inv_freq = 1.0 / (ROPE_BASE ** (np.arange(HALF, dtype=np.float64) * 2.0 / ROPE))
    ang = np.arange(L, dtype=np.float64)[:, None] * inv_freq[None, :]  # [L, 32]
    cosT = np.cos(ang).T.astype(np.float32)  # [32, L]
    sinT = np.sin(ang).T.astype(np.float32)
    cc64 = np.concatenate([cosT, cosT], axis=0)        # [64, L]
    ss64 = np.concatenate([-sinT, sinT], axis=0)       # [64, L]
    CC = np.concatenate([cc64, cc64], axis=0)          # [128, L] (both head slots)
    SS = np.concatenate([ss64, ss64], axis=0)          # [128, L]

    CM = np.zeros((4, 128, BQ), dtype=np.float32)
    for j in range(4):
        for lk in range(128):
            CM[j, lk, j * 128 + lk :] = 1.0

    in_maps = []
    for r in range(N_CORES):
        heads = [2 * r, 2 * r + 1]
        wk_blocks, wq_blocks, wks_blocks, wqs_blocks, wv_blocks = [], [], [], [], []
        for i, h in enumerate(heads):
            kr = W_kr[:, h * ROPE : (h + 1) * ROPE]
            kb = W_uk[:, h * SPLIT : (h + 1) * SPLIT]
            qr = W_qr[:, h * ROPE : (h + 1) * ROPE] * scale
            qb = W_uq[:, h * SPLIT : (h + 1) * SPLIT] * scale
            # head-dim order: slot0 head [rope;base], slot1 head [base;rope]
            # (rope rows land at partitions 0:64 / 64:128 respectively)
            if i == 0:
                wk_blocks.append(np.concatenate([kr, kb], axis=1))
                wq_blocks.append(np.concatenate([qr, qb], axis=1))
            else:
                wk_blocks.append(np.concatenate([kb, kr], axis=1))
                wq_blocks.append(np.concatenate([qb, qr], axis=1))
            # swapped rope halves for the rotate-half term (64 cols per head)
            wks_blocks.append(np.concatenate([kr[:, HALF:], kr[:, :HALF]], axis=1))
            wqs_blocks.append(np.concatenate([qr[:, HALF:], qr[:, :HALF]], axis=1))
            wv_blocks.append(W_uv[:, h * HD : (h + 1) * HD])
        wo_rows = np.concatenate(
            [W_out[h * HD : (h + 1) * HD, :] for h in heads], axis=0
        )
        in_maps.append(
            {
                "xT": bf(xt[:, r * LLOC : (r + 1) * LLOC]),
                "wdkv": bf(W_dkv),
                "wdq": bf(W_dq),
                "wk": bf(np.concatenate(wk_blocks, axis=1)),
                "wq": bf(np.concatenate(wq_blocks, axis=1)),
                "wks": bf(np.concatenate(wks_blocks, axis=1)),
                "wqs": bf(np.concatenate(wqs_blocks, axis=1)),
                "wv": bf(np.concatenate(wv_blocks, axis=1)),
                "wo": bf(wo_rows),
                "CC": bf(CC),
                "SS": bf(SS),
                "CM": bf(CM),
            }
        )
    return in_maps


def _get_program(reps=1):
    if reps not in _CACHE:
        _CACHE[reps] = _build_program(reps)
    return _CACHE[reps]


def make_runner(in_maps, reps=1, empty=False):
    """Persistent compiled runner for timing: returns run_chain(M) that executes
    the program M times back-to-back on device (chained via the output buffer so
    executions serialize), returning wall seconds for the chain."""
    import time as _time
    import jax
    from jax.sharding import Mesh, PartitionSpec, NamedSharding
    from jax.experimental.shard_map import shard_map
    import concourse.bass2jax as bass2jax

    nc = _EMPTY_CACHE.setdefault(0, _build_empty()) if empty else _get_program(reps)
    bass2jax.install_neuronx_cc_hook()
    partition_name = nc.partition_id_tensor.name if nc.partition_id_tensor else None
    in_names, out_names, out_avals, zero_outs = [], [], [], []
    for alloc in nc.m.functions[0].allocations:
        if not isinstance(alloc, mybir.MemoryLocationSet):
            continue
        name = alloc.memorylocations[0].name
        if alloc.kind == "ExternalInput":
            if name != partition_name:
                in_names.append(name)
        elif alloc.kind == "ExternalOutput":
            out_names.append(name)
            shape = tuple(alloc.tensor_shape)
            dtype = mybir.dt.np(alloc.dtype)
            out_avals.append(jax.core.ShapedArray(shape, dtype))
            zero_outs.append(np.zeros(shape, dtype))
    n_params = len(in_names)
    in_names_all = in_names + out_names
    if partition_name is not None:
        in_names_all = in_names_all + [partition_name]

    def _body(*args):
        operands = list(args)
        if partition_name is not None:
            operands.append(bass2jax.partition_id_tensor())
        outs = bass2jax._bass_exec_p.bind(
            *operands,
            out_avals=tuple(out_avals),
            in_names=tuple(in_names_all),
            out_names=tuple(out_names),
            lowering_input_output_aliases=(),
            sim_require_finite=True,
            sim_require_nnan=True,
            nc=nc,
        )
        return tuple(outs)

    devices = jax.devices()[:N_CORES]
    mesh = Mesh(np.asarray(devices), ("core",))
    n_outs = len(out_names)
    in_specs = (PartitionSpec("core"),) * (n_params + n_outs)
    out_specs = (PartitionSpec("core"),) * n_outs
    sharded = jax.jit(
        shard_map(_body, mesh=mesh, in_specs=in_specs, out_specs=out_specs, check_rep=False),
        keep_unused=True,
    )
    sh = NamedSharding(mesh, PartitionSpec("core"))
    concat_in = [
        np.concatenate([np.asarray(in_maps[c][nm]) for c in range(N_CORES)], axis=0)
        for nm in in_names
    ]
    concat_zeros = [
        np.zeros((N_CORES * z.shape[0], *z.shape[1:]), z.dtype) for z in zero_outs
    ]
    dev_in = [jax.device_put(a, sh) for a in concat_in]
    dev_zero = [jax.device_put(a, sh) for a in concat_zeros]
    outs = sharded(*dev_in, *dev_zero)
    jax.block_until_ready(outs)  # compile + warm

    def run_chain(M):
        z = list(dev_zero)
        t0 = _time.perf_counter()
        outs = None
        for _ in range(M):
            outs = sharded(*dev_in, *z)
            z = list(outs)
        jax.block_until_ready(outs)
        return _time.perf_counter() - t0

    return run_chain


_EMPTY_CACHE = {}


def _build_empty():
    """Minimal program with the same I/O signature class: one tiny DMA."""
    nc = bacc.Bacc("TRN2", target_bir_lowering=False, debug=False, num_devices=N_CORES)
    xT = nc.dram_tensor("xT", [128, 128], F32, kind="ExternalInput")
    OUT = nc.dram_tensor("OUT", [128, 128], F32, kind="ExternalOutput")
    with tile.TileContext(nc) as tc:
        with tc.tile_pool(name="sb", bufs=1) as sb:
            t = sb.tile([128, 128], F32, tag="t")
            nc.sync.dma_start(t[:], xT[:])
            nc.sync.dma_start(OUT[:], t[:])
    nc.compile()
    return nc


def make_empty_runner():
    in_maps = [{"xT": np.zeros((128, 128), np.float32)} for _ in range(N_CORES)]
    return make_runner(in_maps, empty=True)


def kernel(x, W_dkv, W_dq, W_uk, W_uv, W_uq, W_qr, W_kr, W_out, b_out, reps=1):
    x = np.asarray(x, dtype=np.float32)
    in_maps = _host_inputs(
        x,
        np.asarray(W_dkv, np.float32), np.asarray(W_dq, np.float32),
        np.asarray(W_uk, np.float32), np.asarray(W_uv, np.float32),
        np.asarray(W_uq, np.float32), np.asarray(W_qr, np.float32),
        np.asarray(W_kr, np.float32), np.asarray(W_out, np.float32),
    )
    nc = _get_program(reps)
    res = run_bass_kernel_spmd(nc, in_maps, core_ids=list(range(N_CORES)), trace=False)
    out = np.zeros((L, D), dtype=np.float32)
    for r in range(N_CORES):
        out += res.results[r]["OUT"]
    out += np.asarray(b_out, np.float32)[None, :]
    return out.reshape(1, L, D)

